# revision 37
# baseline (speedup 1.0000x reference)
"""Trainium2 Bass kernel for nn_NodeAttentionPerMetaPath (GAT-style node attention).

Reference computation (N=8192, F_IN=256, d=64):
    h      = x @ trans                      # [N, d]
    e1     = h @ attn[:d];  e2 = h @ attn[d:]
    scores = leaky_relu(e1 + e2.T, 0.2)     # [N, N]
    masked = where(mask==0, -1e15, scores)
    out    = softmax(masked, axis=1) @ h    # [N, d]

Sharding: rows (r) across 8 cores, 1024 rows each. Every core computes the
full h locally from a streamed copy of x (no collectives).

Algebra (exp of leaky_relu as a max of two exponentials; the exp(a*e1) factor
cancels in the softmax ratio):
    P'[r,j] = m[r,j] * max(C[r]*D[j], 1),  C = exp((1-a)e1), D = exp((1-a)e2)
    out[r]  = (sum_j P'[r,j]*B2[j]*h[j]) / (sum_j P'[r,j]*B2[j]),
    B2 = exp(a*e2); B2*h and B2 live as columns of one lhsT so a single
    accumulated PE matmul yields numerator AND denominator.

Device data flow is [j, r] so NO [N,N] transpose is ever needed on-device:
    - v[j,r] = max(C[r]*D[j], 1): DVE tensor_scalar (4x 16-bit mode)
    - P'T    = v * maskT in place: DVE quad tensor_tensor (packed fp16, 2x).
      ALL elementwise stays on the DVE: the Pool engine shares one SBUF port
      pair with it (lock-held per instruction) so any concurrent Pool op
      stalls DVE perf-mode ops ~15x (measured).
    - out.T  = accumulated PE matmul over 64 j-chunks, lhsT = [B2*h | B2];
      accum batches are emitted AHEAD of the he matmuls in the PE queue
      (lag 4-6 groups) so the PE never stalls on the x DMA.
    - normalize: recip16 = 1024/denom via Ln/Exp on ACT (fp16-normal range;
      raw 1/denom underflows fp16 subnormals), 1/1024 folded into the
      numerator copy; pipelined in r-halves; output fp16.

Host-side packing (lossless or quantization-only input repacking):
    - x -> xT4: fp16, pre-transposed into [g, p, kk, fc, n] 4-chunk groups
    - mask -> maskT16, fp16 0/1, pre-swizzled [g, p, kk, r] so each group is
      one DMA with 128 contiguous 8KB rows (cheap issue)
    - rhs_f = [trans | trans@attn] fp16, pre-swizzled [p, fc, d]
    - per-core chunk rotation: core c sees its OWN 8 node-chunks first (c_rep
      is needed early)
"""

from contextlib import ExitStack

import numpy as np

import concourse.bass as bass
import concourse.bacc as bacc
import concourse.mybir as mybir
import concourse.tile as tile
from concourse.bass_utils import run_bass_kernel_spmd
from concourse.masks import make_identity

f32 = mybir.dt.float32
f16 = mybir.dt.float16

Exp = mybir.ActivationFunctionType.Exp
Ident = mybir.ActivationFunctionType.Identity

N_CORES = 8
N = 8192
F_IN = 256
D = 64  # F_OUT
ALPHA = 0.2

R = N // N_CORES  # own rows per core
JC = N // 128  # j-chunks
FC = F_IN // 128  # f-chunks
KG = 4  # j-chunks per x/he group
NG = JC // KG

# haug columns: 0:64 = B2*h, 64 = B2 (denominator), 65 = zero pad
H_ONE = D
H_W = D + 2
HE_W = D + 2  # he columns: 0:64 h, 64 e1, 65 e2



def build_kernel(ctx: ExitStack, tc: tile.TileContext, xT4, maskT16, rhs_f, outT):
    nc = tc.nc

    singles = ctx.enter_context(tc.tile_pool(name="singles", bufs=1))
    xp = ctx.enter_context(tc.tile_pool(name="xp", bufs=9))
    maskp = ctx.enter_context(tc.tile_pool(name="maskp", bufs=6))
    vp = ctx.enter_context(tc.tile_pool(name="vp", bufs=12))
    ps_he = ctx.enter_context(tc.tile_pool(name="ps_he", bufs=2, space="PSUM"))
    ps_o = ctx.enter_context(tc.tile_pool(name="ps_o", bufs=1, space="PSUM"))
    outp = ctx.enter_context(tc.tile_pool(name="outp", bufs=1))

    # ---- interleaved input streams. own-row x groups 0/1 feed the c_rep
    # critical path: their DMAs go first on the sync queue while the small
    # rhs issues in parallel from the gpsimd queue
    rhs_sb = singles.tile([128, FC, HE_W], f16)
    nc.gpsimd.dma_start(out=rhs_sb, in_=rhs_f)
    x_tiles = []
    m_tiles = {}
    for g in range(2):
        xt = xp.tile([128, KG, FC, 128], f16, tag="x")
        nc.sync.dma_start(out=xt, in_=xT4[g])
        x_tiles.append(xt)
    for g in range(2):
        mt = maskp.tile([128, KG, R], f16, tag="m")
        nc.sync.dma_start(out=mt, in_=maskT16[g])
        m_tiles[g] = mt
    # x issues run two groups ahead of the mask stream so the he matmuls
    # never wait on the x DMA (xp bufs=7 gives the tiles for it)
    def issue_x(g):
        xt = xp.tile([128, KG, FC, 128], f16, tag="x")
        nc.sync.dma_start(out=xt, in_=xT4[g])
        x_tiles.append(xt)

    for g in range(2, 5):
        issue_x(g)
    for g in range(2, NG):
        if g + 3 < NG:
            issue_x(g + 3)
        mt = maskp.tile([128, KG, R], f16, tag="m")
        nc.sync.dma_start(out=mt, in_=maskT16[g])
        m_tiles[g] = mt

    # pin the natural_log_exp_and_others ACT table (id 6) at boot
    nc.scalar.add_instruction(
        mybir.InstLoadActFuncSet(
            name=nc.get_next_instruction_name(), ins=[], outs=[], act_func_set_id=6
        )
    )
    ident = singles.tile([128, 128], f16)
    make_identity(nc, ident)
    ones128 = singles.tile([128, 128], f16)
    nc.vector.memset(ones128, 1.0)
    ones_row = singles.tile([1, D], f16)
    nc.vector.memset(ones_row, 1.0)

    haug = singles.tile([128, JC, H_W], f16)
    nc.vector.memset(haug[:, :, H_ONE + 1], 0.0)
    # f32 per-partition scalars: D (for the tensor_scalar), B2 (ACT scale), C
    scl_d = singles.tile([128, JC], f32)
    scl_b2 = singles.tile([128, JC], f32)
    scl_c = singles.tile([128, 16], f32)
    c_rep = singles.tile([128, R], f16)

    po = ps_o.tile([D + 2, R], f32)

    v_tiles = {}

    def attention_ts(g):
        # v = max(C[r]*D[j], 1): per-chunk tensor_scalar
        v = vp.tile([128, KG, R], f16, tag="v")
        v_tiles[g] = v
        for kk in range(KG):
            k = g * KG + kk
            nc.vector.tensor_scalar(
                v[:, kk, :], c_rep, scl_d[:, k:k + 1], 1.0,
                mybir.AluOpType.mult, mybir.AluOpType.max,
            )

    def attention_tt(g):
        v = v_tiles[g]
        nc.vector.tensor_tensor(v, v, m_tiles[g], mybir.AluOpType.mult)

    first_pe = [True]

    def attention_pe(g, last=False):
        v = v_tiles[g]
        if last:
            # hv-major order: each PSUM half completes as early as possible
            # so the normalize pipeline starts on half 0 while half 1 accums
            for hv in range(2):
                for kk in range(KG):
                    k = g * KG + kk
                    nc.tensor.matmul(
                        po[:, hv * 512:(hv + 1) * 512],
                        haug[:, k, 0:D + 2],
                        v[:, kk, hv * 512:(hv + 1) * 512],
                        start=False,
                        stop=(kk == KG - 1),
                    )
            return
        for kk in range(KG):
            k = g * KG + kk
            # PSUM bank limit: one matmul's output stays within 2KB/partition
            for hv in range(2):
                nc.tensor.matmul(
                    po[:, hv * 512:(hv + 1) * 512],
                    haug[:, k, 0:D + 2],
                    v[:, kk, hv * 512:(hv + 1) * 512],
                    start=first_pe[0],
                    stop=(last and kk == KG - 1),
                )
            first_pe[0] = False

    # ---- per-group pipeline
    pe_done = 0
    pe_order = []
    deferred_he = []
    for g in range(NG):
        # PE accumulation first in the queue: this work is always ready
        # (lagged 4+ groups), so the tensor engine never stalls on the x DMA
        # wait of the he matmuls behind it; 2-group batches keep it in long
        # runs. Draining faster than this starves the he->ACT->TS producer
        # chain behind it in the PE queue (measured +16us).
        while len(pe_order) - pe_done > 4 + 2:
            attention_pe(pe_order[pe_done])
            pe_done += 1
            attention_pe(pe_order[pe_done])
            pe_done += 1
        xt = x_tiles[g]
        he = ps_he.tile([128, KG, HE_W], f32, tag="he")
        for kk in range(KG):
            for fc in range(FC):
                nc.tensor.matmul(
                    he[:, kk, :], xt[:, kk, fc, :], rhs_sb[:, fc, :],
                    start=(fc == 0), stop=(fc == FC - 1),
                )
        ks = slice(g * KG, (g + 1) * KG)

        def act_batches(gg, hee):
            kss = slice(gg * KG, (gg + 1) * KG)
            nc.scalar.activation(
                scl_d[:, kss], hee[:, :, D + 1], Exp, scale=1.0 - ALPHA
            )
            nc.scalar.activation(scl_b2[:, kss], hee[:, :, D + 1], Exp, scale=ALPHA)
            nc.scalar.activation(
                haug[:, kss, H_ONE], hee[:, :, D + 1], Exp, scale=ALPHA
            )
            for kk in range(KG):
                k = gg * KG + kk
                # haug h columns = B2*h (per-partition scale AP)
                nc.scalar.activation(
                    haug[:, k, 0:D], hee[:, kk, 0:D], Ident,
                    scale=scl_b2[:, k:k + 1],
                )

        if g < 2:
            # per-chunk C FIRST: these gate the c_rep diag chain
            for kk in range(KG):
                nc.scalar.activation(
                    scl_c[:, g * KG + kk:g * KG + kk + 1], he[:, kk, D:D + 1],
                    Exp, scale=1.0 - ALPHA,
                )
        act_batches(g, he)

        if g == 1:
            # own chunks 0..7 done -> c_rep[p, r] = C[r] (broadcast across
            # partitions) via diag(C) matmul with an all-ones lhsT
            with tc.tile_pool(name="crep_tmp", bufs=1) as tmp, \
                 tc.tile_pool(name="crep_ps", bufs=1, space="PSUM") as tmps:
                cps = tmps.tile([128, R], f32)
                for rb in range(8):
                    dg = tmp.tile([128, 128], f16, tag="dg", bufs=2)
                    # on DVE: it is idle during the head, and the scalar
                    # queue is saturated with the group-0/1 ACT batches
                    nc.vector.tensor_scalar(
                        dg, ident, scl_c[:, rb:rb + 1], None, mybir.AluOpType.mult
                    )
                    nc.tensor.matmul(
                        cps[:, rb * 128:(rb + 1) * 128], ones128, dg,
                        start=True, stop=True,
                    )
                nc.vector.tensor_copy(c_rep, cps)

            for gg in (0, 1):
                attention_ts(gg)
                attention_tt(gg)
                pe_order.append(gg)
        elif g >= 2:
            attention_ts(g)
            if g == NG - 1:
                # last group: TT split by r-halves; the hv0 accumulation (the
                # gate for the whole normalize chain) starts after only half
                # the TT work, and the hv1 TT overlaps it
                v = v_tiles[g]
                for hv in range(2):
                    s = slice(hv * 512, (hv + 1) * 512)
                    nc.vector.tensor_tensor(
                        v[:, :, s], v[:, :, s], m_tiles[g][:, :, s],
                        mybir.AluOpType.mult,
                    )
                pe_order.append(g)
            else:
                attention_tt(g)
                pe_order.append(g)
    # drain everything but the last group, then the last group hv-major
    while pe_done < len(pe_order) - 1:
        attention_pe(pe_order[pe_done])
        pe_done += 1
    attention_pe(pe_order[-1], last=True)

    # ---- normalize: out.T = numer * (1/denom), pipelined by r-halves.
    # 1/d = exp(-ln(d)) on the scalar engine (denominator is positive);
    # vector.reciprocal measured 6.5us for [1,1024] - too slow.
    # recip16 = 1024/denom: raw 1/denom can be ~1e-6 which is fp16
    # SUBNORMAL (max rel err was 1.4e-2); scaled by 2^10 it is normal.
    # The 2^-10 compensation folds into the numerator copy scale.
    with tc.tile_pool(name="fin_ps", bufs=1, space="PSUM") as fps:
        ln_row = outp.tile([1, R], f32)
        recip16 = outp.tile([1, R], f16)
        ln_k = outp.tile([1, 1], f32)
        nc.vector.memset(ln_k, float(np.log(1024.0)))
        numer = outp.tile([D, R], f32)
        rr = fps.tile([D, R], f32)
        o16 = outp.tile([D, R], f16)
        for hv in range(2):
            s = slice(hv * 512, (hv + 1) * 512)
            nc.scalar.activation(
                ln_row[:, s], po[D:D + 1, s], mybir.ActivationFunctionType.Ln
            )
            nc.scalar.activation(
                recip16[:, s], ln_row[:, s], Exp, scale=-1.0, bias=ln_k
            )
            nc.scalar.activation(
                numer[:, s], po[0:D, s], mybir.ActivationFunctionType.Copy,
                scale=1.0 / 1024.0,
            )
            nc.tensor.matmul(
                rr[:, s], ones_row, recip16[:, s], start=True, stop=True,
            )
            # one PSUM operand max per tensor_tensor
            nc.vector.tensor_tensor(
                o16[:, s], numer[:, s], rr[:, s], mybir.AluOpType.mult
            )
            nc.sync.dma_start(out=outT[:, s], in_=o16[:, s])


def build_nc():
    nc = bacc.Bacc("TRN2", num_devices=N_CORES)
    xT4 = nc.dram_tensor("xT4", [NG, 128, KG, FC, 128], f16, kind="ExternalInput")
    maskT16 = nc.dram_tensor("maskT16", [NG, 128, KG, R], f16, kind="ExternalInput")
    rhs_f = nc.dram_tensor("rhs_f", [128, FC, HE_W], f16, kind="ExternalInput")
    outT = nc.dram_tensor("outT", [D, R], f16, kind="ExternalOutput")
    with ExitStack() as ctx:
        tc = ctx.enter_context(tile.TileContext(nc))
        build_kernel(
            ctx, tc, xT4[:, :, :, :, :], maskT16[:, :, :, :],
            rhs_f[:, :, :], outT[:, :],
        )
    nc.compile()
    return nc


LAST_RESULTS = None


def kernel(x, mask, trans, attn, _trace=False):
    x = np.asarray(x, dtype=np.float32)
    mask = np.asarray(mask)
    trans = np.asarray(trans, dtype=np.float32)
    attn = np.asarray(attn, dtype=np.float32)

    x16 = np.ascontiguousarray(x, dtype=np.float16)
    # fused weights: h plus e1/e2 from one matmul ([trans | trans@a1 | trans@a2])
    ta12 = trans @ np.concatenate([attn[:D], attn[D:]], axis=1)  # [F_IN, 2]
    rhs_flat = np.concatenate([trans, ta12], axis=1).astype(np.float16)
    # device layout [p, c, d]: partition p holds input-feature c*128+p
    rhs_f = np.ascontiguousarray(
        rhs_flat.reshape(FC, 128, HE_W).transpose(1, 0, 2)
    )
    maskT = np.ascontiguousarray(mask.T, dtype=np.float16)  # [N(j), N(r)] 0/1

    nc = build_nc()
    in_maps = []
    xc = x16.reshape(JC, 128, F_IN)
    mc = maskT.reshape(NG, KG, 128, N)  # [g, kk, p, r_full]
    for c in range(N_CORES):
        # chunk rotation: own 8 chunks first, then the rest in order
        order = list(range(c * 8, c * 8 + 8)) + [
            k for k in range(JC) if not (c * 8 <= k < c * 8 + 8)
        ]
        # xT4[g][p][kk][fc][n] = x[chunk(4g+kk) node n, fc*128+p]
        xr = xc[order]  # [JC, 128(n), F_IN]
        xT4 = np.ascontiguousarray(
            xr.reshape(NG, KG, 128, FC, 128).transpose(0, 4, 1, 3, 2)
        )
        # mask per group, swizzled [g, p, kk, r]: row per partition is
        # KG*R contiguous
        mg = maskT.reshape(JC, 128, N)[order][:, :, c * R:(c + 1) * R]
        mg = mg.reshape(NG, KG, 128, R).transpose(0, 2, 1, 3)  # [g, p, kk, r]
        m16 = np.ascontiguousarray(mg)
        in_maps.append({
            "xT4": xT4,
            "maskT16": m16,
            "rhs_f": rhs_f,
        })
    res = run_bass_kernel_spmd(nc, in_maps, list(range(N_CORES)), trace=_trace)
    global LAST_RESULTS
    LAST_RESULTS = res
    out = np.concatenate(
        [res.results[c]["outT"].astype(np.float32).T for c in range(N_CORES)], axis=0
    )
    return np.ascontiguousarray(out, dtype=np.float32)


if __name__ == "__main__":
    nc = build_nc()
    print("built OK")
